# revision 36
# baseline (speedup 1.0000x reference)
"""Trainium2 Bass kernel for a message-aggregation (single-query attention) block.

Computation (per batch row b):
    Q = A @ Wq.T ; K = M @ Wk.T ; V = M @ Wv.T
    attn = softmax(Q . K / sqrt(D))
    out = sigmoid(A @ Wg.T + bg) * LN(attn-weighted V @ Wo.T + bo)

Host-side algebraic restructuring (exact up to fp reassociation):
    scores[b,n] = A[b] @ (Wq.T @ Wk / sqrt(D)) @ M[b,n].T
    agg[b]      = (sum_n attn[b,n] M[b,n]) @ (Wo @ Wv).T + bo
so K and V are never materialized; the device makes a single streaming pass
over `messages` plus small 512x512 matmuls.

All streamed data is bf16 (host-downcast): halves HBM traffic and enables the
DVE 2x packed mode; accumulations stay f32. End-to-end rel err ~9e-3 (gate 2e-2).

Softmax normalization is deferred through the LayerNorm: LN is invariant to a
per-row scale, so the unnormalized sum U = sum_n exp(s_n) M_n feeds
agg' = U @ (Wo Wv).T + Z*bo = Z * agg and LN(Z*agg) == LN(agg).

Device dataflow per 128-row batch tile, pipelined per 8-message group:
  - q8 = Q-tile replicated 8x (scalar engine) so the score products are a
    flat [P,4096]x[P,4096] tensor_tensor (broadcast APs force 1x; flat packed
    bf16 runs 2x)
  - per group: product (DVE 2x) -> bf16 fold tree (adds, ~2x) -> small [P,8]
    1x reduce (reduce-class ops are 1x-only on DVE, so shrink first) -> exp
    (scalar) -> diag(exp s) chunk (GPSIMD) -> 8 PE matmuls accumulate U in PSUM
  - tail: transpose, @ (Wo Wv).T, +Z*bo, LayerNorm, * gate (GPSIMD; DVE for
    the final tile), DMA out

Sharding: pure data parallel over the batch dim across 8 cores; the small
512x512 weights are replicated.
"""

import math
from contextlib import ExitStack

import numpy as np

import concourse.bacc as bacc
import concourse.bass as bass
import concourse.mybir as mybir
import concourse.tile as tile
from concourse.bass_utils import run_bass_kernel_spmd
from concourse.masks import make_identity

B = 4096
N = 32
D = 512
NCORES = 8
BLOC = B // NCORES  # 512
P = 128
NT = BLOC // P  # 4 batch tiles per core
KT = D // P  # 4 contraction tiles
NH = 16  # messages per DMA half-tile
NG = 4  # dot groups per tile
GM = N // NG  # 8 messages per group
SCALE = math.sqrt(D)
LN_EPS = 1e-5

F32 = mybir.dt.float32
BF16 = mybir.dt.bfloat16
ALU = mybir.AluOpType
ACTF = mybir.ActivationFunctionType


def broadcast_mid(ap2d, count):
    """[P, D] AP -> [P, count, D] AP with a step-0 middle dim."""
    return bass.AP(
        tensor=ap2d.tensor,
        offset=ap2d.offset,
        ap=[ap2d.ap[0], [0, count], ap2d.ap[1]],
    )


def broadcast_last(ap2d, count):
    """[P, C] AP -> [P, C, count] AP with a step-0 last dim."""
    return bass.AP(
        tensor=ap2d.tensor,
        offset=ap2d.offset,
        ap=[ap2d.ap[0], ap2d.ap[1], [0, count]],
    )


def build_program():
    nc = bacc.Bacc(
        "TRN2",
        target_bir_lowering=False,
        debug=False,
        num_devices=NCORES,
    )

    m_d = nc.dram_tensor("m", [BLOC, N, D], BF16, kind="ExternalInput")
    at_d = nc.dram_tensor("at", [D, BLOC], BF16, kind="ExternalInput")  # A.T
    wqk_d = nc.dram_tensor("wqk", [D, D], BF16, kind="ExternalInput")  # Wq.T Wk/sqrtD
    wgt_d = nc.dram_tensor("wgt", [D, D], BF16, kind="ExternalInput")  # Wg.T
    wvo_d = nc.dram_tensor("wvo", [D, D], BF16, kind="ExternalInput")  # (Wo @ Wv).T
    ones_d = nc.dram_tensor("ones", [1, D], BF16, kind="ExternalInput")
    bg_d = nc.dram_tensor("bg", [1, D], BF16, kind="ExternalInput")
    bo_d = nc.dram_tensor("bo", [1, D], BF16, kind="ExternalInput")
    gamma_d = nc.dram_tensor("gamma", [1, D], BF16, kind="ExternalInput")
    beta_d = nc.dram_tensor("beta", [1, D], BF16, kind="ExternalInput")
    out_d = nc.dram_tensor("out", [BLOC, D], F32, kind="ExternalOutput")

    with tile.TileContext(nc) as tc, ExitStack() as ctx:
        consts = ctx.enter_context(tc.tile_pool(name="consts", bufs=1))
        atp = ctx.enter_context(tc.tile_pool(name="atp", bufs=KT))
        wts = ctx.enter_context(tc.tile_pool(name="wts", bufs=3 * KT))
        q8p = ctx.enter_context(tc.tile_pool(name="q8p", bufs=NT))
        ggp = ctx.enter_context(tc.tile_pool(name="ggp", bufs=NT))
        mpool = ctx.enter_context(tc.tile_pool(name="mpool", bufs=5))
        scp = ctx.enter_context(tc.tile_pool(name="scp", bufs=2))
        expp = ctx.enter_context(tc.tile_pool(name="expp", bufs=2))
        diagp = ctx.enter_context(tc.tile_pool(name="diagp", bufs=3))
        scrp = ctx.enter_context(tc.tile_pool(name="scrp", bufs=1))
        smalls = ctx.enter_context(tc.tile_pool(name="smalls", bufs=2))
        tailp = ctx.enter_context(tc.tile_pool(name="tailp", bufs=2))
        outp = ctx.enter_context(tc.tile_pool(name="outp", bufs=2))
        ps_a = ctx.enter_context(tc.tile_pool(name="ps_a", bufs=2, space="PSUM"))
        ps_b = ctx.enter_context(tc.tile_pool(name="ps_b", bufs=2, space="PSUM"))
        ps_t = ctx.enter_context(tc.tile_pool(name="ps_t", bufs=2, space="PSUM"))

        # ---- constants + DMA order on the sync queue ----------------------
        ident = consts.tile([P, P], F32)
        make_identity(nc, ident[:])
        ident_bf = consts.tile([P, P], BF16)
        nc.scalar.copy(ident_bf[:], ident[:])

        ones_row = consts.tile([1, D], BF16)
        nc.sync.dma_start(out=ones_row[:], in_=ones_d[:, :])
        bg_row = consts.tile([1, D], BF16)
        nc.sync.dma_start(out=bg_row[:], in_=bg_d[:, :])
        bo_row = consts.tile([1, D], BF16)
        nc.sync.dma_start(out=bo_row[:], in_=bo_d[:, :])
        gamma_row = consts.tile([1, D], BF16)
        nc.sync.dma_start(out=gamma_row[:], in_=gamma_d[:, :])
        beta_row = consts.tile([1, D], BF16)
        nc.sync.dma_start(out=beta_row[:], in_=beta_d[:, :])

        eps_t = consts.tile([P, 1], F32)
        nc.vector.memset(eps_t[:], LN_EPS)
        zeros_t = consts.tile([P, 1], F32)
        nc.vector.memset(zeros_t[:], 0.0)

        at_t = []
        for k in range(KT):
            t = atp.tile([P, BLOC], BF16, tag="at")
            nc.sync.dma_start(out=t[:], in_=at_d[k * P : (k + 1) * P, :])
            at_t.append(t)
        wqk_t = []
        for k in range(KT):
            t = wts.tile([P, D], BF16, tag="w")
            nc.sync.dma_start(out=t[:], in_=wqk_d[k * P : (k + 1) * P, :])
            wqk_t.append(t)

        mh = []  # mh[i][h] = [P, NH, D] bf16
        wgt_t = []
        wvo_t = []

        def emit_mdma(i):
            pair = []
            for h in range(2):
                t = mpool.tile([P, NH, D], BF16, tag="m")
                nc.sync.dma_start(
                    out=t[:],
                    in_=m_d[i * P : (i + 1) * P, h * NH : (h + 1) * NH, :],
                )
                pair.append(t)
            mh.append(pair)

        emit_mdma(0)
        for k in range(KT):
            t = wts.tile([P, D], BF16, tag="w")
            nc.sync.dma_start(out=t[:], in_=wgt_d[k * P : (k + 1) * P, :])
            wgt_t.append(t)
        emit_mdma(1)
        for k in range(KT):
            t = wts.tile([P, D], BF16, tag="w")
            nc.sync.dma_start(out=t[:], in_=wvo_d[k * P : (k + 1) * P, :])
            wvo_t.append(t)
        emit_mdma(2)
        emit_mdma(3)

        # ---- phase 1a: Qt = A @ Wqk (compact); q8 replication is spread
        # into the tile loop so tile 0's copies don't queue behind all four.
        qt_t = []
        for m in range(NT):
            pq = ps_a.tile([P, D], F32, tag="psa")
            for k in range(KT):
                nc.tensor.matmul(
                    pq[:],
                    lhsT=at_t[k][:, m * P : (m + 1) * P],
                    rhs=wqk_t[k][:],
                    start=(k == 0),
                    stop=(k == KT - 1),
                )
            qt = q8p.tile([P, D], BF16, tag="qt")
            nc.scalar.copy(qt[:], pq[:])
            qt_t.append(qt)

        q8_t = [None] * NT

        def emit_q8(m, eng=None):
            # DVE tensor_copy runs 4x (used for tile 0 so the first dots
            # don't wait on the scalar queue); scalar engine otherwise.
            q8 = q8p.tile([P, GM, D], BF16, tag="q8")
            for c in range(GM):
                if eng is nc.vector:
                    nc.vector.tensor_copy(q8[:, c, :], qt_t[m][:])
                else:
                    nc.scalar.copy(q8[:, c, :], qt_t[m][:])
            q8_t[m] = q8

        # replicate gamma/beta/bo rows across partitions via K=1 matmuls
        # (their copies must precede the sigmoids on the scalar queue,
        # since gg/gb consume them right after each sigmoid)
        reps = []
        for nm, row in (("gam", gamma_row), ("bet", beta_row), ("bor", bo_row)):
            pr = ps_t.tile([P, D], F32, tag="pst")
            nc.tensor.matmul(
                pr[:], lhsT=ones_row[:, 0:P], rhs=row[:], start=True, stop=True
            )
            rep = consts.tile([P, D], BF16, tag=nm)
            nc.scalar.copy(rep[:], pr[:])
            reps.append(rep)
        gamma_rep, beta_rep, bo_rep = reps

        # ---- phase 1b: gate = sigmoid(A @ Wg.T + bg); gg/gb on gpsimd -----
        gg_t = []
        gb_t = []
        for m in range(NT):
            pg = ps_b.tile([P, D], F32, tag="psb")
            for k in range(KT):
                nc.tensor.matmul(
                    pg[:],
                    lhsT=at_t[k][:, m * P : (m + 1) * P],
                    rhs=wgt_t[k][:],
                    start=(k == 0),
                    stop=False,
                )
            nc.tensor.matmul(
                pg[:],
                lhsT=ones_row[:, 0:P],
                rhs=bg_row[:],
                start=False,
                stop=True,
            )
            gate = smalls.tile([P, D], BF16, tag="gate")
            nc.scalar.activation(gate[:], pg[:], ACTF.Sigmoid)
            gg = ggp.tile([P, D], BF16, tag="gg")
            nc.gpsimd.tensor_mul(gg[:], gate[:], gamma_rep[:])
            gg_t.append(gg)
            gb = ggp.tile([P, D], BF16, tag="gb")
            nc.gpsimd.tensor_mul(gb[:], gate[:], beta_rep[:])
            gb_t.append(gb)

        # product scratches (persistent; same-engine WAW is free).
        # Group 0's product gets its own buffer: the scalar engine reduces it
        # (accumulator path) while DVE overwrites `prod` for groups 1-3.
        prod = scrp.tile([P, GM, D], BF16, tag="prod")
        prod_a = scrp.tile([P, GM, D], BF16, tag="prod_a")
        junk = scrp.tile([P, D], BF16, tag="junk")

        # ---- steady state ------------------------------------------------
        def emit_tile(i):
            """Dots (flat 2x products; reduces split DVE/scalar-accumulator)
            -> one exp over [P,32] with fused Z -> diags (gpsimd, 2 chunks)
            -> 32 PE matmuls accumulating U[i] in PSUM."""
            sc = scp.tile([P, N], F32, tag="sc")
            # group 0: DVE product, scalar-engine accumulator does the reduces
            mt_0 = mh[i][0][:, 0:GM, :]
            nc.vector.tensor_mul(prod_a[:], mt_0, q8_t[i][:])
            for j in range(GM):
                nc.scalar.activation(
                    junk[:],
                    prod_a[:, j, :],
                    ACTF.Identity,
                    accum_out=sc[:, j : j + 1],
                )
            # groups 1-3: DVE product + DVE reduce
            for g in range(1, NG):
                n0 = g * GM
                h, off = divmod(n0, NH)
                mt_g = mh[i][h][:, off : off + GM, :]
                nc.vector.tensor_mul(prod[:], mt_g, q8_t[i][:])
                nc.vector.tensor_reduce(
                    sc[:, n0 : n0 + GM], prod[:], axis=mybir.AxisListType.X, op=ALU.add
                )
            # exp + diag + PE accumulation per half-tile: the second half's
            # weights are still being reduced while the first half's matmuls
            # stream, halving the post-dots serial tail.
            expd = expp.tile([P, N], F32, tag="expd")
            zh = smalls.tile([P, 2], F32, tag="zh")
            pm = ps_a.tile([P, D], F32, tag="psa")
            for half in range(2):
                n0 = half * NH
                nc.scalar.activation(
                    expd[:, n0 : n0 + NH],
                    sc[:, n0 : n0 + NH],
                    ACTF.Exp,
                    bias=zeros_t[:, 0:1],
                    accum_out=zh[:, half : half + 1],
                )
                dg = diagp.tile([P, NH, P], BF16, tag="diag")
                nc.gpsimd.tensor_mul(
                    dg[:],
                    broadcast_mid(ident_bf[:], NH),
                    broadcast_last(expd[:, n0 : n0 + NH], P),
                )
                for j in range(NH):
                    n = n0 + j
                    nc.tensor.matmul(
                        pm[:],
                        lhsT=dg[:, j, :],
                        rhs=mh[i][half][:, j, :],
                        start=(n == 0),
                        stop=(n == N - 1),
                    )
            z = smalls.tile([P, 1], F32, tag="z")
            nc.vector.tensor_add(z[:], zh[:, 0:1], zh[:, 1:2])
            return z, pm

        def emit_tail(i, z, pm, last=False):
            magg = tailp.tile([P, D], BF16, tag="magg")
            nc.scalar.copy(magg[:], pm[:])

            pt = ps_t.tile([P, KT, P], BF16, tag="pst2")
            for j in range(KT):
                nc.tensor.transpose(
                    pt[:, j, :], magg[:, j * P : (j + 1) * P], ident_bf[:]
                )
            maggT = tailp.tile([P, KT, P], BF16, tag="maggT")
            nc.scalar.copy(maggT[:], pt[:])

            # agg' = U @ (Wo Wv).T (+ Z*bo below); LN absorbs the Z scale
            pa = ps_b.tile([P, D], F32, tag="psb")
            for j in range(KT):
                nc.tensor.matmul(
                    pa[:],
                    lhsT=maggT[:, j, :],
                    rhs=wvo_t[j][:],
                    start=(j == 0),
                    stop=(j == KT - 1),
                )
            aggs = tailp.tile([P, D], F32, tag="aggs")
            nc.vector.scalar_tensor_tensor(
                out=aggs[:],
                in0=bo_rep[:],
                scalar=z[:, 0:1],
                in1=pa[:],
                op0=ALU.mult,
                op1=ALU.add,
            )

            # LayerNorm over d
            stats = smalls.tile([P, nc.vector.BN_STATS_DIM], F32, tag="stats")
            nc.vector.bn_stats(stats[:], aggs[:])
            mv = smalls.tile([P, nc.vector.BN_AGGR_DIM], F32, tag="mv")
            nc.vector.bn_aggr(mv[:], stats[:])
            sq = smalls.tile([P, 1], F32, tag="sq")
            nc.scalar.activation(sq[:], mv[:, 1:2], ACTF.Sqrt, bias=eps_t[:, 0:1])
            rstd = smalls.tile([P, 1], F32, tag="rstd")
            nc.vector.reciprocal(rstd[:], sq[:])
            negmr = smalls.tile([P, 1], F32, tag="negmr")
            nc.vector.tensor_scalar(
                negmr[:],
                mv[:, 0:1],
                scalar1=rstd[:, 0:1],
                scalar2=-1.0,
                op0=ALU.mult,
                op1=ALU.mult,
            )
            normed = outp.tile([P, D], BF16, tag="normed")
            nc.scalar.activation(
                normed[:], aggs[:], ACTF.Identity, bias=negmr[:, 0:1], scale=rstd[:, 0:1]
            )

            # out = (gate*gamma)*normed + gate*beta
            # (gpsimd mid-stream; DVE for the last tile, where DVE is idle
            # and the slower gpsimd would gate the final store)
            eng = nc.vector if last else nc.gpsimd
            o = outp.tile([P, D], F32, tag="out")
            eng.tensor_mul(o[:], normed[:], gg_t[i][:])
            eng.tensor_add(o[:], o[:], gb_t[i][:])
            nc.sync.dma_start(out=out_d[i * P : (i + 1) * P, :], in_=o[:])

        emit_q8(0, eng=nc.vector)
        pending = None  # (i, z, pm)
        for i in range(NT):
            if i + 1 < NT:
                emit_q8(i + 1)
            if pending is not None:
                emit_tail(*pending)
            z, pm = emit_tile(i)
            pending = (i, z, pm)
        emit_tail(*pending, last=True)

    nc.compile()
    return nc


_CACHED_NC = None


def _get_program():
    global _CACHED_NC
    if _CACHED_NC is None:
        _CACHED_NC = build_program()
    return _CACHED_NC


def make_in_maps(agent_hidden, messages, Wq, Wk, Wv, Wo, bo, gamma, beta, Wg, bg):
    import ml_dtypes

    bf16 = ml_dtypes.bfloat16
    A = np.asarray(agent_hidden, np.float32)
    M = np.asarray(messages, np.float32).astype(bf16)
    wq = np.asarray(Wq, np.float64)
    wk = np.asarray(Wk, np.float64)
    wv = np.asarray(Wv, np.float64)
    wo = np.asarray(Wo, np.float64)
    wg = np.asarray(Wg, np.float32)

    wqk = np.ascontiguousarray(((wq.T @ wk) / SCALE).astype(bf16))
    wvo = np.ascontiguousarray((wo @ wv).T.astype(bf16))
    wgt = np.ascontiguousarray(wg.T.astype(bf16))
    bg_r = np.ascontiguousarray(np.asarray(bg, np.float32).reshape(1, D).astype(bf16))
    bo_r = np.ascontiguousarray(np.asarray(bo, np.float32).reshape(1, D).astype(bf16))
    gamma_r = np.ascontiguousarray(
        np.asarray(gamma, np.float32).reshape(1, D).astype(bf16)
    )
    beta_r = np.ascontiguousarray(
        np.asarray(beta, np.float32).reshape(1, D).astype(bf16)
    )
    ones_r = np.ones((1, D), bf16)

    in_maps = []
    for c in range(NCORES):
        sl = slice(c * BLOC, (c + 1) * BLOC)
        in_maps.append(
            {
                "m": np.ascontiguousarray(M[sl]),
                "at": np.ascontiguousarray(A[sl].T.astype(bf16)),
                "wqk": wqk,
                "wgt": wgt,
                "wvo": wvo,
                "ones": ones_r,
                "bg": bg_r,
                "bo": bo_r,
                "gamma": gamma_r,
                "beta": beta_r,
            }
        )
    return in_maps


def kernel(**inputs) -> np.ndarray:
    nc = _get_program()
    in_maps = make_in_maps(**inputs)
    res = run_bass_kernel_spmd(nc, in_maps, core_ids=list(range(NCORES)))
    return np.concatenate([r["out"] for r in res.results], axis=0)


# revision 48
# speedup vs baseline: 1.0954x; 1.0954x over previous
"""Trainium2 Bass kernel for a message-aggregation (single-query attention) block.

Computation (per batch row b):
    Q = A @ Wq.T ; K = M @ Wk.T ; V = M @ Wv.T
    attn = softmax(Q . K / sqrt(D))
    out = sigmoid(A @ Wg.T + bg) * LN(attn-weighted V @ Wo.T + bo)

Host-side algebraic restructuring (exact up to fp reassociation):
    scores[b,n] = A[b] @ (Wq.T @ Wk / sqrt(D)) @ M[b,n].T
    agg[b]      = (sum_n attn[b,n] M[b,n]) @ (Wo @ Wv).T + bo
so K and V are never materialized; the device makes a single streaming pass
over `messages` plus small 512x512 matmuls.

All streamed data is bf16 (host-downcast): halves HBM traffic and enables the
DVE 2x packed mode; accumulations stay f32. End-to-end rel err ~9e-3 (gate 2e-2).

Softmax normalization is deferred through the LayerNorm: LN is invariant to a
per-row scale, so the unnormalized sum U = sum_n exp(s_n) M_n feeds
agg' = U @ (Wo Wv).T + Z*bo = Z * agg and LN(Z*agg) == LN(agg).

Device dataflow per 128-row batch tile (tails pipelined one tile behind):
  - q8 = Q-tile replicated 8x (scalar engine, interleaved one tile ahead) so
    each score product is a flat [P,4096]x[P,4096] tensor_tensor (broadcast
    APs force the DVE to 1x; flat packed bf16 runs 2x)
  - scores in four 8-message groups: group 0's per-message reduces run on the
    scalar engine's accumulator (activation accum_out) from a dedicated
    product buffer, groups 1-3 are DVE product + 1x [P,8,512] reduce
  - one exp over [P,32] with fused Z accumulation -> diag(exp s) chunks
    (GPSIMD) -> 32 PE matmuls accumulate U in PSUM
  - tail: transpose, @ (Wo Wv).T, +Z*bo, LayerNorm, * gate (GPSIMD; DVE for
    the final tile), DMA out

Sharding: pure data parallel over the batch dim across 8 cores; the small
512x512 weights are replicated.
"""

import math
from contextlib import ExitStack

import numpy as np

import concourse.bacc as bacc
import concourse.bass as bass
import concourse.mybir as mybir
import concourse.tile as tile
from concourse.bass_utils import run_bass_kernel_spmd
from concourse.masks import make_identity

B = 4096
N = 32
D = 512
NCORES = 8
BLOC = B // NCORES  # 512
P = 128
NT = BLOC // P  # 4 batch tiles per core
KT = D // P  # 4 contraction tiles
NH = 16  # messages per DMA half-tile
NG = 4  # dot groups per tile
GM = N // NG  # 8 messages per group
SCALE = math.sqrt(D)
LN_EPS = 1e-5

F32 = mybir.dt.float32
BF16 = mybir.dt.bfloat16
ALU = mybir.AluOpType
ACTF = mybir.ActivationFunctionType


def broadcast_mid(ap2d, count):
    """[P, D] AP -> [P, count, D] AP with a step-0 middle dim."""
    return bass.AP(
        tensor=ap2d.tensor,
        offset=ap2d.offset,
        ap=[ap2d.ap[0], [0, count], ap2d.ap[1]],
    )


def broadcast_last(ap2d, count):
    """[P, C] AP -> [P, C, count] AP with a step-0 last dim."""
    return bass.AP(
        tensor=ap2d.tensor,
        offset=ap2d.offset,
        ap=[ap2d.ap[0], ap2d.ap[1], [0, count]],
    )


def build_program():
    nc = bacc.Bacc(
        "TRN2",
        target_bir_lowering=False,
        debug=False,
        num_devices=NCORES,
    )

    m_d = nc.dram_tensor("m", [BLOC, N, D], BF16, kind="ExternalInput")
    at_d = nc.dram_tensor("at", [D, BLOC], BF16, kind="ExternalInput")  # A.T
    wqk_d = nc.dram_tensor("wqk", [D, D], BF16, kind="ExternalInput")  # Wq.T Wk/sqrtD
    wgt_d = nc.dram_tensor("wgt", [D, D], BF16, kind="ExternalInput")  # Wg.T
    wvo_d = nc.dram_tensor("wvo", [D, D], BF16, kind="ExternalInput")  # (Wo @ Wv).T
    ones_d = nc.dram_tensor("ones", [1, D], BF16, kind="ExternalInput")
    bg_d = nc.dram_tensor("bg", [1, D], BF16, kind="ExternalInput")
    bo_d = nc.dram_tensor("bo", [1, D], BF16, kind="ExternalInput")
    gamma_d = nc.dram_tensor("gamma", [1, D], BF16, kind="ExternalInput")
    beta_d = nc.dram_tensor("beta", [1, D], BF16, kind="ExternalInput")
    out_d = nc.dram_tensor("out", [BLOC, D], F32, kind="ExternalOutput")

    with tile.TileContext(nc) as tc, ExitStack() as ctx:
        consts = ctx.enter_context(tc.tile_pool(name="consts", bufs=1))
        atp = ctx.enter_context(tc.tile_pool(name="atp", bufs=KT))
        wts = ctx.enter_context(tc.tile_pool(name="wts", bufs=3 * KT))
        q8p = ctx.enter_context(tc.tile_pool(name="q8p", bufs=NT))
        ggp = ctx.enter_context(tc.tile_pool(name="ggp", bufs=NT))
        mpool = ctx.enter_context(tc.tile_pool(name="mpool", bufs=5))
        scp = ctx.enter_context(tc.tile_pool(name="scp", bufs=2))
        expp = ctx.enter_context(tc.tile_pool(name="expp", bufs=2))
        diagp = ctx.enter_context(tc.tile_pool(name="diagp", bufs=3))
        scrp = ctx.enter_context(tc.tile_pool(name="scrp", bufs=1))
        smalls = ctx.enter_context(tc.tile_pool(name="smalls", bufs=2))
        tailp = ctx.enter_context(tc.tile_pool(name="tailp", bufs=2))
        outp = ctx.enter_context(tc.tile_pool(name="outp", bufs=2))
        ps_a = ctx.enter_context(tc.tile_pool(name="ps_a", bufs=2, space="PSUM"))
        ps_b = ctx.enter_context(tc.tile_pool(name="ps_b", bufs=2, space="PSUM"))
        ps_t = ctx.enter_context(tc.tile_pool(name="ps_t", bufs=2, space="PSUM"))

        # ---- constants + DMA order on the sync queue ----------------------
        ident = consts.tile([P, P], F32)
        make_identity(nc, ident[:])
        ident_bf = consts.tile([P, P], BF16)
        nc.scalar.copy(ident_bf[:], ident[:])

        eps_t = consts.tile([P, 1], F32)
        nc.vector.memset(eps_t[:], LN_EPS)
        zeros_t = consts.tile([P, 1], F32)
        nc.vector.memset(zeros_t[:], 0.0)

        at_t = []
        for k in range(KT):
            t = atp.tile([P, BLOC], BF16, tag="at")
            nc.sync.dma_start(out=t[:], in_=at_d[k * P : (k + 1) * P, :])
            at_t.append(t)
        wqk_t = []
        for k in range(KT):
            t = wts.tile([P, D], BF16, tag="w")
            nc.sync.dma_start(out=t[:], in_=wqk_d[k * P : (k + 1) * P, :])
            wqk_t.append(t)

        mh = []  # mh[i][h] = [P, NH, D] bf16
        wgt_t = []
        wvo_t = []

        def emit_mdma(i):
            pair = []
            for h in range(2):
                t = mpool.tile([P, NH, D], BF16, tag="m")
                nc.sync.dma_start(
                    out=t[:],
                    in_=m_d[i * P : (i + 1) * P, h * NH : (h + 1) * NH, :],
                )
                pair.append(t)
            mh.append(pair)

        emit_mdma(0)
        # constant rows after M(0): their 5 dispatch slots would otherwise
        # delay the ramp-critical at/wqk/M(0) transfers; consumers (rep
        # matmuls, gate bias, tails) all run much later
        ones_row = consts.tile([1, D], BF16)
        nc.sync.dma_start(out=ones_row[:], in_=ones_d[:, :])
        bg_row = consts.tile([1, D], BF16)
        nc.sync.dma_start(out=bg_row[:], in_=bg_d[:, :])
        bo_row = consts.tile([1, D], BF16)
        nc.sync.dma_start(out=bo_row[:], in_=bo_d[:, :])
        gamma_row = consts.tile([1, D], BF16)
        nc.sync.dma_start(out=gamma_row[:], in_=gamma_d[:, :])
        beta_row = consts.tile([1, D], BF16)
        nc.sync.dma_start(out=beta_row[:], in_=beta_d[:, :])
        for k in range(KT):
            t = wts.tile([P, D], BF16, tag="w")
            nc.sync.dma_start(out=t[:], in_=wgt_d[k * P : (k + 1) * P, :])
            wgt_t.append(t)
        emit_mdma(1)
        for k in range(KT):
            t = wts.tile([P, D], BF16, tag="w")
            nc.sync.dma_start(out=t[:], in_=wvo_d[k * P : (k + 1) * P, :])
            wvo_t.append(t)
        emit_mdma(2)
        emit_mdma(3)

        # ---- phase 1a: Qt = A @ Wqk (compact); q8 replication is spread
        # into the tile loop so tile 0's copies don't queue behind all four.
        qt_t = []
        q8_t = [None] * NT
        for m in range(NT):
            pq = ps_a.tile([P, D], F32, tag="psa")
            for k in range(KT):
                nc.tensor.matmul(
                    pq[:],
                    lhsT=at_t[k][:, m * P : (m + 1) * P],
                    rhs=wqk_t[k][:],
                    start=(k == 0),
                    stop=(k == KT - 1),
                )
            if m == 0:
                # tile 0's replicated Q comes straight from PSUM -- it gates
                # the very first products, so skip the compact-qt round trip
                q8 = q8p.tile([P, GM, D], BF16, tag="q8")
                for c in range(GM):
                    nc.scalar.copy(q8[:, c, :], pq[:])
                q8_t[0] = q8
                qt_t.append(None)
            else:
                qt = q8p.tile([P, D], BF16, tag="qt")
                nc.scalar.copy(qt[:], pq[:])
                qt_t.append(qt)

        def emit_q8(m):
            q8 = q8p.tile([P, GM, D], BF16, tag="q8")
            for c in range(GM):
                nc.scalar.copy(q8[:, c, :], qt_t[m][:])
            q8_t[m] = q8

        # replicate gamma/beta/bo rows across partitions via K=1 matmuls
        # (their copies must precede the sigmoids on the scalar queue,
        # since gg/gb consume them right after each sigmoid)
        reps = []
        for nm, row in (("gam", gamma_row), ("bet", beta_row), ("bor", bo_row)):
            pr = ps_t.tile([P, D], F32, tag="pst")
            nc.tensor.matmul(
                pr[:], lhsT=ones_row[:, 0:P], rhs=row[:], start=True, stop=True
            )
            rep = consts.tile([P, D], BF16, tag=nm)
            nc.scalar.copy(rep[:], pr[:])
            reps.append(rep)
        gamma_rep, beta_rep, bo_rep = reps

        # ---- phase 1b: gate = sigmoid(A @ Wg.T + bg); gg/gb on gpsimd -----
        gg_t = []
        gb_t = []
        for m in range(NT):
            pg = ps_b.tile([P, D], F32, tag="psb")
            for k in range(KT):
                nc.tensor.matmul(
                    pg[:],
                    lhsT=at_t[k][:, m * P : (m + 1) * P],
                    rhs=wgt_t[k][:],
                    start=(k == 0),
                    stop=False,
                )
            nc.tensor.matmul(
                pg[:],
                lhsT=ones_row[:, 0:P],
                rhs=bg_row[:],
                start=False,
                stop=True,
            )
            gate = smalls.tile([P, D], BF16, tag="gate")
            nc.scalar.activation(gate[:], pg[:], ACTF.Sigmoid)
            gg = ggp.tile([P, D], BF16, tag="gg")
            nc.gpsimd.tensor_mul(gg[:], gate[:], gamma_rep[:])
            gg_t.append(gg)
            gb = ggp.tile([P, D], BF16, tag="gb")
            nc.gpsimd.tensor_mul(gb[:], gate[:], beta_rep[:])
            gb_t.append(gb)

        # product scratches (persistent; same-engine WAW is free).
        # Group 0's product gets its own buffer: the scalar engine reduces it
        # (accumulator path) while DVE overwrites `prod` for groups 1-3.
        prod = scrp.tile([P, GM, D], BF16, tag="prod")
        prod_a = scrp.tile([P, GM, D], BF16, tag="prod_a")
        prod_b = scrp.tile([P, GM, D], BF16, tag="prod_b")
        junk = scrp.tile([P, D], BF16, tag="junk")

        # ---- steady state ------------------------------------------------
        def emit_tile(i):
            """Dots (flat 2x products; reduces split DVE/scalar-accumulator)
            -> one exp over [P,32] with fused Z -> diags (gpsimd, 2 chunks)
            -> 32 PE matmuls accumulating U[i] in PSUM."""
            sc = scp.tile([P, N], F32, tag="sc")
            # group 0: DVE product, scalar-engine accumulator does the reduces
            mt_0 = mh[i][0][:, 0:GM, :]
            nc.vector.tensor_mul(prod_a[:], mt_0, q8_t[i][:])
            for j in range(GM):
                nc.scalar.activation(
                    junk[:],
                    prod_a[:, j, :],
                    ACTF.Identity,
                    accum_out=sc[:, j : j + 1],
                )
            # group 1: DVE product into its own buffer; the scalar engine
            # reduces the first half, DVE the second
            mt_1 = mh[i][0][:, GM : 2 * GM, :]
            nc.vector.tensor_mul(prod_b[:], mt_1, q8_t[i][:])
            for j in range(GM // 2):
                nc.scalar.activation(
                    junk[:],
                    prod_b[:, j, :],
                    ACTF.Identity,
                    accum_out=sc[:, GM + j : GM + j + 1],
                )
            nc.vector.tensor_reduce(
                sc[:, GM + GM // 2 : 2 * GM],
                prod_b[:, GM // 2 : GM, :],
                axis=mybir.AxisListType.X,
                op=ALU.add,
            )
            # groups 2-3: DVE product + DVE reduce
            for g in range(2, NG):
                n0 = g * GM
                h, off = divmod(n0, NH)
                mt_g = mh[i][h][:, off : off + GM, :]
                nc.vector.tensor_mul(prod[:], mt_g, q8_t[i][:])
                nc.vector.tensor_reduce(
                    sc[:, n0 : n0 + GM], prod[:], axis=mybir.AxisListType.X, op=ALU.add
                )
            expd = expp.tile([P, N], F32, tag="expd")
            z = smalls.tile([P, 1], F32, tag="z")
            nc.scalar.activation(
                expd[:], sc[:], ACTF.Exp, bias=zeros_t[:, 0:1], accum_out=z[:]
            )
            pm = ps_a.tile([P, D], F32, tag="psa")
            for half in range(2):
                n0 = half * NH
                dg = diagp.tile([P, NH, P], BF16, tag="diag")
                nc.gpsimd.tensor_mul(
                    dg[:],
                    broadcast_mid(ident_bf[:], NH),
                    broadcast_last(expd[:, n0 : n0 + NH], P),
                )
                for j in range(NH):
                    n = n0 + j
                    nc.tensor.matmul(
                        pm[:],
                        lhsT=dg[:, j, :],
                        rhs=mh[i][half][:, j, :],
                        start=(n == 0),
                        stop=(n == N - 1),
                    )
            return z, pm

        def emit_tail(i, z, pm, last=False):
            magg = tailp.tile([P, D], BF16, tag="magg")
            nc.scalar.copy(magg[:], pm[:])

            pt = ps_t.tile([P, KT, P], BF16, tag="pst2")
            for j in range(KT):
                nc.tensor.transpose(
                    pt[:, j, :], magg[:, j * P : (j + 1) * P], ident_bf[:]
                )
            maggT = tailp.tile([P, KT, P], BF16, tag="maggT")
            nc.scalar.copy(maggT[:], pt[:])

            # agg' = U @ (Wo Wv).T (+ Z*bo below); LN absorbs the Z scale
            pa = ps_b.tile([P, D], F32, tag="psb")
            for j in range(KT):
                nc.tensor.matmul(
                    pa[:],
                    lhsT=maggT[:, j, :],
                    rhs=wvo_t[j][:],
                    start=(j == 0),
                    stop=(j == KT - 1),
                )
            aggs = tailp.tile([P, D], F32, tag="aggs")
            nc.vector.scalar_tensor_tensor(
                out=aggs[:],
                in0=bo_rep[:],
                scalar=z[:, 0:1],
                in1=pa[:],
                op0=ALU.mult,
                op1=ALU.add,
            )

            # LayerNorm over d
            stats = smalls.tile([P, nc.vector.BN_STATS_DIM], F32, tag="stats")
            nc.vector.bn_stats(stats[:], aggs[:])
            mv = smalls.tile([P, nc.vector.BN_AGGR_DIM], F32, tag="mv")
            nc.vector.bn_aggr(mv[:], stats[:])
            sq = smalls.tile([P, 1], F32, tag="sq")
            nc.scalar.activation(sq[:], mv[:, 1:2], ACTF.Sqrt, bias=eps_t[:, 0:1])
            rstd = smalls.tile([P, 1], F32, tag="rstd")
            nc.vector.reciprocal(rstd[:], sq[:])
            negmr = smalls.tile([P, 1], F32, tag="negmr")
            nc.vector.tensor_scalar(
                negmr[:],
                mv[:, 0:1],
                scalar1=rstd[:, 0:1],
                scalar2=-1.0,
                op0=ALU.mult,
                op1=ALU.mult,
            )
            normed = outp.tile([P, D], BF16, tag="normed")
            nc.scalar.activation(
                normed[:], aggs[:], ACTF.Identity, bias=negmr[:, 0:1], scale=rstd[:, 0:1]
            )

            # out = (gate*gamma)*normed + gate*beta
            # (gpsimd mid-stream; DVE for the last tile, where DVE is idle
            # and the slower gpsimd would gate the final store)
            eng = nc.vector if last else nc.gpsimd
            o = outp.tile([P, D], F32, tag="out")
            eng.tensor_mul(o[:], normed[:], gg_t[i][:])
            eng.tensor_add(o[:], o[:], gb_t[i][:])
            nc.sync.dma_start(out=out_d[i * P : (i + 1) * P, :], in_=o[:])

        pending = None  # (i, z, pm)
        for i in range(NT):
            if i + 1 < NT:
                emit_q8(i + 1)
            if pending is not None:
                emit_tail(*pending)
            z, pm = emit_tile(i)
            pending = (i, z, pm)
        emit_tail(*pending, last=True)

    nc.compile()
    return nc


_CACHED_NC = None


def _get_program():
    global _CACHED_NC
    if _CACHED_NC is None:
        _CACHED_NC = build_program()
    return _CACHED_NC


def make_in_maps(agent_hidden, messages, Wq, Wk, Wv, Wo, bo, gamma, beta, Wg, bg):
    import ml_dtypes

    bf16 = ml_dtypes.bfloat16
    A = np.asarray(agent_hidden, np.float32)
    M = np.asarray(messages, np.float32).astype(bf16)
    wq = np.asarray(Wq, np.float64)
    wk = np.asarray(Wk, np.float64)
    wv = np.asarray(Wv, np.float64)
    wo = np.asarray(Wo, np.float64)
    wg = np.asarray(Wg, np.float32)

    wqk = np.ascontiguousarray(((wq.T @ wk) / SCALE).astype(bf16))
    wvo = np.ascontiguousarray((wo @ wv).T.astype(bf16))
    wgt = np.ascontiguousarray(wg.T.astype(bf16))
    bg_r = np.ascontiguousarray(np.asarray(bg, np.float32).reshape(1, D).astype(bf16))
    bo_r = np.ascontiguousarray(np.asarray(bo, np.float32).reshape(1, D).astype(bf16))
    gamma_r = np.ascontiguousarray(
        np.asarray(gamma, np.float32).reshape(1, D).astype(bf16)
    )
    beta_r = np.ascontiguousarray(
        np.asarray(beta, np.float32).reshape(1, D).astype(bf16)
    )
    ones_r = np.ones((1, D), bf16)

    in_maps = []
    for c in range(NCORES):
        sl = slice(c * BLOC, (c + 1) * BLOC)
        in_maps.append(
            {
                "m": np.ascontiguousarray(M[sl]),
                "at": np.ascontiguousarray(A[sl].T.astype(bf16)),
                "wqk": wqk,
                "wgt": wgt,
                "wvo": wvo,
                "ones": ones_r,
                "bg": bg_r,
                "bo": bo_r,
                "gamma": gamma_r,
                "beta": beta_r,
            }
        )
    return in_maps


def kernel(**inputs) -> np.ndarray:
    nc = _get_program()
    in_maps = make_in_maps(**inputs)
    res = run_bass_kernel_spmd(nc, in_maps, core_ids=list(range(NCORES)))
    return np.concatenate([r["out"] for r in res.results], axis=0)
